# revision 15
# baseline (speedup 1.0000x reference)
"""Trainium2 Bass kernel for nn_MinervaEnhancedLossV3.

Contract: kernel(**inputs) takes FULL unsharded inputs (B=2048), shards
batch-wise across 8 NeuronCores, runs one SPMD Bass program, and combines
per-batch partial statistics on the host into the scalar loss.

Device algorithm (per core, B_pc=264 padded batches = 22 groups of 12):
  layout: "group tiles" [120, 2304] with partition p = c*12 + b_local
          (channel-outer), free axis = H*W positions.
  E  = fp16(exp(pred))                          ACT
  masked_e = (t_rep == iota_c) * E              DVE scalar_tensor_tensor
             (accum col -> per-(c,b) presence sums -> unique colors)
  e_pv[b,pos]  = sum_c masked_e   -> packed PSUM   PE (block one-hot lhsT)
  sumexp[b,pos]= sum_c E          -> packed PSUM   PE
  ce = log(sumexp) - log(e_pv)                  ACT logs + DVE sub
  pt = exp(-ce); u = relu(1-pt); p25 = exp(2.5*log(u))   ACT
  fsum[b] = sum_pos p25*ce                      DVE STT accum
  geq = [E >= e_pv_rep]; g_cnt = sum_c geq      DVE TT + PE
  eq = [g_cnt == 1]; eq_cnt, iou = accums       DVE
  transitions / [t==inputs] counts: int compares on packed tiles.
Host: focal weights w(unique, transitions), ultra_teal, exact bonus,
  copy-penalty (necessary-condition guard + numpy resolve of the ~dozen
  candidate batches), bonuses, nan/inf guard.

All per-batch accumulator columns are staged in one SBUF tile and written
out with a single DMA per core.
"""

import os
from contextlib import ExitStack

import numpy as np

import concourse.bass as bass
import concourse.bacc as bacc
import concourse.tile as tile
import concourse.mybir as mybir
from concourse.bass_utils import run_bass_kernel_spmd

F16 = mybir.dt.float16
F32 = mybir.dt.float32
I32 = mybir.dt.int32
AF = mybir.ActivationFunctionType
OP = mybir.AluOpType
X_AXIS = mybir.AxisListType.X

N_CORES = 8
B_FULL = 2048
C = 10
H = W = 48
HW = H * W                      # 2304
BG = 12                         # batches per group
P = BG * C                      # 120 partitions per group tile
MM_N = 512                      # matmul free-dim chunk

LAST_EXEC_NS = None


def _cfg(b_pc):
    """supergroup sizes + psum D-chunks for a padded per-core batch count"""
    ng = b_pc // BG
    sgs = []
    left = ng
    while left > 0:
        sgs.append(min(10, left))
        left -= sgs[-1]
    d_chunks = [(0, 1024), (1024, 1024), (2048, 256)]
    return ng, sgs, d_chunks


def _spatial_weights():
    cy, cx = H // 2, W // 2
    yy = np.arange(H, dtype=np.float64)[:, None]
    xx = np.arange(W, dtype=np.float64)[None, :]
    dist = np.sqrt((yy - cy) ** 2 + (xx - cx) ** 2)
    md = np.sqrt((H // 2) ** 2 + (W // 2) ** 2)
    return (1.0 + 0.3 * (1.0 - dist / md)).astype(np.float32)   # [H, W]


class ColMap:
    """column index allocator for the staged accumulator tile"""

    def __init__(self):
        self.n = 0
        self.m = {}

    def col(self, name):
        if name not in self.m:
            self.m[name] = self.n
            self.n += 1
        return self.m[name]


def build_nc(b_pc, finalize=True):
    ng, sg_sizes, d_chunks = _cfg(b_pc)
    n_sg = len(sg_sizes)
    nc = bacc.Bacc(trn_type="TRN2") if finalize else bass.Bass(trn_type="TRN2")

    pred_in = nc.dram_tensor("pred_in", [b_pc, C, HW], F32, kind="ExternalInput")
    t_in = nc.dram_tensor("t_in", [b_pc, HW], I32, kind="ExternalInput")
    i_in = nc.dram_tensor("i_in", [b_pc, HW], I32, kind="ExternalInput")
    strat_in = nc.dram_tensor("strat_in", [128, 512], F32, kind="ExternalInput")

    cm = ColMap()
    for sg in range(n_sg):
        for ph in range(len(d_chunks)):
            cm.col(f"fs_{sg}_{ph}")
            cm.col(f"iou_{sg}_{ph}")
            cm.col(f"eqc_{sg}_{ph}")
        cm.col(f"th_{sg}")
        cm.col(f"tv_{sg}")
        cm.col(f"d_{sg}")
    for g in range(ng):
        for k in range(3):
            cm.col(f"cnt3_{g}_{k}")
    ncols = cm.n

    out_cols = nc.dram_tensor("out_cols", [P, ncols], F32, kind="ExternalOutput")
    out_strat = nc.dram_tensor("out_strat", [128, 1], F32, kind="ExternalOutput")

    # ---- inline constants ----
    sw = np.repeat(_spatial_weights().reshape(1, HW), P, axis=0).astype(np.float16)
    sw_const = nc.inline_tensor(sw, name="sw_const")                     # [P, HW]
    iota = (np.arange(P) % C).astype(np.float32).reshape(P, 1)
    iota_const = nc.inline_tensor(iota, name="iota_const")               # [P, 1]
    # channel-sum one-hot lhsT: L[p=(b,c), gl*P + gl*BG+b] = 1  (k=group tile
    # partitions, cols = packed row q = gl*BG+b)
    lhs = np.zeros((P, 10 * P), dtype=np.float16)
    # broadcast lhsT: B[k=q=(gl*BG+b), gl*P + b*C+c] = 1 (k=packed rows,
    # cols = group tile partition p)
    bca = np.zeros((P, 10 * P), dtype=np.float16)
    for gl in range(10):
        for b in range(BG):
            for c in range(C):
                lhs[b * C + c, gl * P + gl * BG + b] = 1.0
                bca[gl * BG + b, gl * P + b * C + c] = 1.0
    lhs_const = nc.inline_tensor(lhs, name="lhs_const")                  # [P, 10*P]
    bca_const = nc.inline_tensor(bca, name="bca_const")                  # [P, 10*P]

    with tile.TileContext(nc) as tc, ExitStack() as es:
        _emit(es, tc, nc, cm, sg_sizes, d_chunks,
              pred_in, t_in, i_in, strat_in, out_cols, out_strat,
              sw_const, iota_const, lhs_const, bca_const)
    if finalize:
        nc.finalize()
    return nc, cm, sg_sizes, d_chunks


def _emit(es, tc, nc, cm, sg_sizes, d_chunks,
          pred_in, t_in, i_in, strat_in, out_cols, out_strat,
          sw_const, iota_const, lhs_const, bca_const):
    dma = nc.sync.dma_start
    stt = nc.vector.scalar_tensor_tensor

    singles = es.enter_context(tc.tile_pool(name="singles", bufs=1))
    xpool = es.enter_context(tc.tile_pool(name="xpool", bufs=2))
    epool = es.enter_context(tc.tile_pool(name="epool", bufs=1))
    mpool = es.enter_context(tc.tile_pool(name="mpool", bufs=1))
    tpool = es.enter_context(tc.tile_pool(name="tpool", bufs=1))
    small = es.enter_context(tc.tile_pool(name="small", bufs=1))
    scr = es.enter_context(tc.tile_pool(name="scr", bufs=2))
    psum = es.enter_context(tc.tile_pool(name="psum", bufs=1, space="PSUM"))

    # resident constants + staged output columns
    sw_t = singles.tile([P, HW], F16)
    dma(out=sw_t[:], in_=sw_const[:, :])
    iota_t = singles.tile([P, 1], F32)
    dma(out=iota_t[:], in_=iota_const[:, :])
    lhs_t = singles.tile([P, 10 * P], F16)
    dma(out=lhs_t[:], in_=lhs_const[:, :])
    bca_t = singles.tile([P, 10 * P], F16)
    dma(out=bca_t[:], in_=bca_const[:, :])
    colstage = singles.tile([P, cm.n], F32)
    nc.vector.memset(colstage[:], 0.0)
    # absorb const-DMA queue sems into the DVE vector clock so downstream
    # compute ops need <=2 sync waits (walrus STT struct limit)
    absorb = singles.tile([P, 4], F32, tag="absorb")
    nc.vector.tensor_copy(absorb[:, 0:1], iota_t[:, 0:1])
    nc.vector.tensor_copy(absorb[:, 1:2], sw_t[:, 0:1])
    nc.vector.tensor_copy(absorb[:, 2:3], lhs_t[:, 0:1])
    nc.vector.tensor_copy(absorb[:, 3:4], bca_t[:, 0:1])

    # strategic partial sums
    strat_sb = singles.tile([128, 512], F32, tag="strat")
    dma(out=strat_sb[:], in_=strat_in[:, :])
    strat_col = singles.tile([128, 1], F32, tag="strat_col")
    nc.vector.reduce_sum(strat_col[:], strat_sb[:], axis=X_AXIS)
    dma(out=out_strat[:, :], in_=strat_col[:])

    def ccol(name, r):
        return colstage[:r, cm.col(name):cm.col(name) + 1]

    g0 = 0
    for sg, G in enumerate(sg_sizes):
        R = G * BG                                   # packed rows this supergroup
        # ---- packed int tiles: targets / inputs ----
        t_pk = tpool.tile([P, HW], I32, tag="t_pk")
        i_pk = tpool.tile([P, HW], I32, tag="i_pk")
        dma(out=t_pk[:R], in_=t_in[g0 * BG:g0 * BG + R, :])
        dma(out=i_pk[:R], in_=i_in[g0 * BG:g0 * BG + R, :])
        t16 = tpool.tile([P, HW], F16, tag="t16")
        nc.vector.tensor_copy(t16[:R], t_pk[:R])

        # transitions (horizontal / vertical) + [t==inputs] count
        scratch = scr.tile([P, HW], F16, tag="scratch")
        t3 = t_pk.rearrange("p (h w) -> p h w", h=H)
        s3 = scratch.rearrange("p (h w) -> p h w", h=H)
        stt(out=s3[:R, :, 0:W - 1], in0=t3[:R, :, 0:W - 1], scalar=0.0,
            in1=t3[:R, :, 1:W], op0=OP.bypass, op1=OP.not_equal,
            accum_out=ccol(f"th_{sg}", R))
        stt(out=s3[:R, 0:H - 1, :], in0=t3[:R, 0:H - 1, :], scalar=0.0,
            in1=t3[:R, 1:H, :], op0=OP.bypass, op1=OP.not_equal,
            accum_out=ccol(f"tv_{sg}", R))
        stt(out=scratch[:R], in0=t_pk[:R], scalar=0.0, in1=i_pk[:R],
            op0=OP.bypass, op1=OP.is_equal, accum_out=ccol(f"d_{sg}", R))

        # ---- per-group: load, exp, PE-broadcast targets, mask ----
        e_tiles = []
        m_tiles = []
        for gl in range(G):
            g = g0 + gl
            x_t = xpool.tile([P, HW], F32, tag="x")
            src = pred_in[g * BG:(g + 1) * BG, :, :].rearrange("b c s -> (b c) s")
            dma(out=x_t[:], in_=src)
            e_t = epool.tile([P, HW], F16, tag=f"e{gl}")
            nc.scalar.activation(e_t[:], x_t[:], AF.Exp)

            me_t = mpool.tile([P, HW], F16, tag=f"m{gl}")
            bcl = bca_t[0:R, gl * P:(gl + 1) * P]
            for h0 in range(0, HW, 1024):
                hn = min(1024, HW - h0)
                trep = psum.tile([P, 1024], F32, tag="bcast")
                for k0 in range(0, hn, MM_N):
                    kn = min(MM_N, hn - k0)
                    nc.tensor.matmul(
                        trep[:, k0:k0 + kn], bcl,
                        t16[0:R, h0 + k0:h0 + k0 + kn],
                        start=True, stop=True)
                stt(out=me_t[:, h0:h0 + hn], in0=trep[:, :hn],
                    scalar=iota_t[:], in1=e_t[:, h0:h0 + hn],
                    op0=OP.is_equal, op1=OP.mult,
                    accum_out=ccol(f"cnt3_{g}_{h0 // 1024}", P))
            e_tiles.append(e_t)
            m_tiles.append(me_t)

        # ---- per D-chunk phase: PE sums, packed math ----
        for ph, (d0, D) in enumerate(d_chunks):
            epv_ps = psum.tile([P, D], F32, tag="epv")
            sum_ps = psum.tile([P, D], F32, tag="sum")
            for gl in range(G):
                lw = lhs_t[:, gl * P:gl * P + R]
                for k0 in range(0, D, MM_N):
                    kn = min(MM_N, D - k0)
                    nc.tensor.matmul(
                        epv_ps[0:R, k0:k0 + kn], lw,
                        m_tiles[gl][:, d0 + k0:d0 + k0 + kn],
                        start=(gl == 0), stop=(gl == G - 1))
                    nc.tensor.matmul(
                        sum_ps[0:R, k0:k0 + kn], lw,
                        e_tiles[gl][:, d0 + k0:d0 + k0 + kn],
                        start=(gl == 0), stop=(gl == G - 1))

            log_s = small.tile([P, 1024], F32, tag="log_s")
            nc.scalar.activation(log_s[:R, :D], sum_ps[0:R], AF.Ln)
            log_pv = small.tile([P, 1024], F32, tag="log_pv")
            nc.scalar.activation(log_pv[:R, :D], epv_ps[0:R], AF.Ln)
            epv16 = small.tile([P, 1024], F16, tag="epv16")
            nc.vector.tensor_copy(epv16[:R, :D], epv_ps[0:R])

            gcnt_ps = psum.tile([P, D], F32, tag="gcnt")
            for gl in range(G):
                rep = psum.tile([P, 1024], F32, tag="bcast")
                bcl = bca_t[0:R, gl * P:(gl + 1) * P]
                for k0 in range(0, D, MM_N):
                    kn = min(MM_N, D - k0)
                    nc.tensor.matmul(
                        rep[:, k0:k0 + kn], bcl, epv16[0:R, k0:k0 + kn],
                        start=True, stop=True)
                geq = scr.tile([P, 1024], F16, tag="geq")
                nc.vector.tensor_tensor(
                    out=geq[:, :D], in0=e_tiles[gl][:, d0:d0 + D],
                    in1=rep[:, :D], op=OP.is_ge)
                lw = lhs_t[:, gl * P:gl * P + R]
                for k0 in range(0, D, MM_N):
                    kn = min(MM_N, D - k0)
                    nc.tensor.matmul(
                        gcnt_ps[0:R, k0:k0 + kn], lw, geq[:, k0:k0 + kn],
                        start=(gl == 0), stop=(gl == G - 1))

            eq = small.tile([P, 1024], F16, tag="eq")
            nc.vector.tensor_scalar(
                out=eq[:R, :D], in0=gcnt_ps[0:R], scalar1=1.0, scalar2=None,
                op0=OP.is_equal, op1=OP.add, accum_out=ccol(f"eqc_{sg}_{ph}", R))
            iou_scr = small.tile([P, 1024], F16, tag="iou_scr")
            stt(out=iou_scr[:R, :D], in0=eq[:R, :D], scalar=0.0,
                in1=sw_t[:R, d0:d0 + D], op0=OP.bypass, op1=OP.mult,
                accum_out=ccol(f"iou_{sg}_{ph}", R))

            ce = small.tile([P, 1024], F32, tag="ce")
            nc.vector.tensor_tensor(out=ce[:R, :D], in0=log_s[:R, :D],
                                    in1=log_pv[:R, :D], op=OP.subtract)
            pt = small.tile([P, 1024], F32, tag="pt")
            nc.scalar.activation(pt[:R, :D], ce[:R, :D], AF.Exp, scale=-1.0)
            u = small.tile([P, 1024], F32, tag="u")
            nc.scalar.activation(u[:R, :D], pt[:R, :D], AF.Relu,
                                 bias=1.0, scale=-1.0)
            lu = small.tile([P, 1024], F32, tag="lu")
            nc.scalar.activation(lu[:R, :D], u[:R, :D], AF.Ln)
            p25 = small.tile([P, 1024], F32, tag="p25")
            nc.scalar.activation(p25[:R, :D], lu[:R, :D], AF.Exp, scale=2.5)
            fs_scr = small.tile([P, 1024], F32, tag="fs_scr")
            stt(out=fs_scr[:R, :D], in0=p25[:R, :D], scalar=0.0,
                in1=ce[:R, :D], op0=OP.bypass, op1=OP.mult,
                accum_out=ccol(f"fs_{sg}_{ph}", R))

        g0 += G

    dma(out=out_cols[:, :], in_=colstage[:])


_NC_CACHE = {}


def _get_nc(b_pc):
    if b_pc not in _NC_CACHE:
        _NC_CACHE[b_pc] = build_nc(b_pc)
    return _NC_CACHE[b_pc]


def _combine(res_list, cm, sg_sizes, d_chunks, b_pc, bpc_real,
             pred, targets, inputs_arr, sf, ps, rd):
    """host-side final reduction from per-core column outputs"""
    ng = b_pc // BG
    n_sg = len(sg_sizes)
    B = pred.shape[0]
    n_cores = len(res_list)

    fsum = np.zeros(B, np.float64)
    iou_s = np.zeros(B, np.float64)
    eqc = np.zeros(B, np.float64)
    uniq = np.zeros(B, np.int64)
    trans = np.zeros(B, np.int64)
    dcnt = np.zeros(B, np.int64)

    for core, r in enumerate(res_list):
        cols = r["out_cols"]                        # [P, ncols]
        sl0 = core * bpc_real
        g0 = 0
        for sg, G in enumerate(sg_sizes):
            R = G * BG
            rows = np.arange(R)
            gb = g0 * BG + rows                     # per-core padded batch idx
            valid = gb < bpc_real
            bidx = sl0 + gb[valid]
            fs = np.zeros(R); io = np.zeros(R); ec = np.zeros(R)
            for ph in range(len(d_chunks)):
                fs += cols[:R, cm.col(f"fs_{sg}_{ph}")]
                io += cols[:R, cm.col(f"iou_{sg}_{ph}")]
                ec += cols[:R, cm.col(f"eqc_{sg}_{ph}")]
            fsum[bidx] = fs[valid]
            iou_s[bidx] = io[valid]
            eqc[bidx] = ec[valid]
            trans[bidx] = np.rint(cols[:R, cm.col(f"th_{sg}")]
                                  + cols[:R, cm.col(f"tv_{sg}")])[valid]
            dcnt[bidx] = np.rint(cols[:R, cm.col(f"d_{sg}")])[valid]
            g0 += G
        # presence: cnt3 columns, rows p=(b,c)
        for g in range(ng):
            cnt = sum(cols[:, cm.col(f"cnt3_{g}_{k}")] for k in range(3))
            pres = (cnt.reshape(BG, C) > 0).sum(1)             # [b]
            gb = g * BG + np.arange(BG)
            valid = gb < bpc_real
            uniq[sl0 + gb[valid]] = pres[valid]

    sw64 = _spatial_weights().astype(np.float64)
    SW = sw64.sum()
    w = np.where(uniq > 4, 1.3, 1.0) * np.where(trans > W, 1.2, 1.0)
    focal = (fsum * w).sum() / (B * HW)

    strict = eqc == HW
    iou = iou_s / SW
    ut = 0.85 * iou + 0.15 * strict
    ut_mean = ut.mean()
    exact_bonus = max(-ut_mean * 5.0, -5.0)

    cand = np.where(eqc == dcnt)[0]
    copy = np.zeros(B, np.float64)
    if cand.size:
        pr = pred.reshape(B, C, HW)
        am = pr[cand].argmax(1)
        copy[cand] = (am == inputs_arr.reshape(B, HW)[cand]).all(1)
    transform_penalty = copy.mean() * 0.5

    strat_total = sum(float(r["out_strat"].sum()) for r in res_list)
    sf_mean = strat_total / sf.size
    creativity = 1.0 / (1.0 + np.exp(-sf_mean)) * 0.1
    strategic = ps.astype(np.float64).mean() * 0.1
    multi = rd.astype(np.float64).mean() * 0.1
    complexity = ut_mean * (HW / 1225.0) * 0.1

    total = (focal + transform_penalty + exact_bonus
             - creativity - strategic - multi - complexity)
    if np.isnan(total) or np.isinf(total):
        total = min(focal, 10.0)
    return np.float32(total)


def kernel(pred, strategic_features, planning_score, reasoning_depth,
           targets, inputs):
    global LAST_EXEC_NS
    pred = np.ascontiguousarray(np.asarray(pred, dtype=np.float32))
    targets = np.ascontiguousarray(np.asarray(targets, dtype=np.int32))
    inputs_arr = np.ascontiguousarray(np.asarray(inputs, dtype=np.int32))
    sf = np.asarray(strategic_features, dtype=np.float32)
    ps = np.asarray(planning_score, dtype=np.float32)
    rd = np.asarray(reasoning_depth, dtype=np.float32)

    B = pred.shape[0]
    bpc = B // N_CORES                                 # 256
    b_pc = ((bpc + BG - 1) // BG) * BG
    if (b_pc // BG) % 2:                               # keep 22 groups for 256
        pass
    nc, cm, sg_sizes, d_chunks = _get_nc(b_pc if bpc % BG == 0 else bpc + (BG - bpc % BG))
    b_pc = bpc + (BG - bpc % BG) % BG

    in_maps = []
    for core in range(N_CORES):
        sl = slice(core * bpc, (core + 1) * bpc)
        p_c = pred[sl].reshape(bpc, C, HW)
        t_c = targets[sl].reshape(bpc, HW)
        i_c = inputs_arr[sl].reshape(bpc, HW)
        pad = b_pc - bpc
        if pad:
            p_c = np.concatenate([p_c, np.broadcast_to(p_c[:1], (pad, C, HW))], 0)
            t_c = np.concatenate([t_c, np.broadcast_to(t_c[:1], (pad, HW))], 0)
            i_c = np.concatenate([i_c, np.broadcast_to(i_c[:1], (pad, HW))], 0)
        in_maps.append({
            "pred_in": np.ascontiguousarray(p_c),
            "t_in": np.ascontiguousarray(t_c),
            "i_in": np.ascontiguousarray(i_c),
            "strat_in": np.ascontiguousarray(sf[sl].reshape(128, 512)),
        })

    trace = os.environ.get("BASSLOSS_TRACE", "0") == "1"
    res = run_bass_kernel_spmd(nc, in_maps, list(range(N_CORES)), trace=trace)
    LAST_EXEC_NS = res.exec_time_ns

    return _combine(res.results, cm, sg_sizes, d_chunks, b_pc, bpc,
                    pred, targets, inputs_arr, sf, ps, rd)


if __name__ == "__main__":
    d = np.load("/root/problem/inputs_cache.npz")
    out = kernel(**{k: d[k] for k in d.files})
    print("kernel out:", out, " exec_ns:", LAST_EXEC_NS)


# revision 20
# speedup vs baseline: 50.8917x; 50.8917x over previous
"""Trainium2 Bass kernel for nn_MinervaEnhancedLossV3.

Contract: kernel(**inputs) takes FULL unsharded inputs (B=2048), shards
batch-wise across 8 NeuronCores, runs one SPMD Bass program, and combines
per-batch partial statistics on the host into the scalar loss.

Device algorithm (per core, B_pc=264 padded batches = 22 groups of 12):
  layout: "group tiles" [120, 2304] with partition p = c*12 + b_local
          (channel-outer), free axis = H*W positions.
  E  = fp16(exp(pred))                          ACT
  masked_e = (t_rep == iota_c) * E              DVE scalar_tensor_tensor
             (accum col -> per-(c,b) presence sums -> unique colors)
  e_pv[b,pos]  = sum_c masked_e   -> packed PSUM   PE (block one-hot lhsT)
  sumexp[b,pos]= sum_c E          -> packed PSUM   PE
  ce = log(sumexp) - log(e_pv)                  ACT logs + DVE sub
  pt = exp(-ce); u = relu(1-pt); p25 = exp(2.5*log(u))   ACT
  fsum[b] = sum_pos p25*ce                      DVE STT accum
  geq = [E >= e_pv_rep]; g_cnt = sum_c geq      DVE TT + PE
  eq = [g_cnt == 1]; eq_cnt, iou = accums       DVE
  transitions / [t==inputs] counts: int compares on packed tiles.
Host: focal weights w(unique, transitions), ultra_teal, exact bonus,
  copy-penalty (necessary-condition guard + numpy resolve of the ~dozen
  candidate batches), bonuses, nan/inf guard.

All per-batch accumulator columns are staged in one SBUF tile and written
out with a single DMA per core.
"""

import os
from contextlib import ExitStack

import numpy as np

import concourse.bass as bass
import concourse.bacc as bacc
import concourse.tile as tile
import concourse.mybir as mybir
from concourse.bass_utils import run_bass_kernel_spmd

F16 = mybir.dt.float16
F32 = mybir.dt.float32
I32 = mybir.dt.int32
AF = mybir.ActivationFunctionType
OP = mybir.AluOpType
X_AXIS = mybir.AxisListType.X

N_CORES = 8
B_FULL = 2048
C = 10
H = W = 48
HW = H * W                      # 2304
BG = 12                         # batches per group
P = BG * C                      # 120 partitions per group tile
MM_N = 512                      # matmul free-dim chunk

LAST_EXEC_NS = None


def _cfg(b_pc):
    """supergroup sizes + psum D-chunks for a padded per-core batch count"""
    ng = b_pc // BG
    sgs = []
    left = ng
    while left > 0:
        sgs.append(min(10, left))
        left -= sgs[-1]
    d_chunks = [(0, 1024), (1024, 1024), (2048, 256)]
    return ng, sgs, d_chunks


def _spatial_weights():
    cy, cx = H // 2, W // 2
    yy = np.arange(H, dtype=np.float64)[:, None]
    xx = np.arange(W, dtype=np.float64)[None, :]
    dist = np.sqrt((yy - cy) ** 2 + (xx - cx) ** 2)
    md = np.sqrt((H // 2) ** 2 + (W // 2) ** 2)
    return (1.0 + 0.3 * (1.0 - dist / md)).astype(np.float32)   # [H, W]


class ColMap:
    """column index allocator for the staged accumulator tile"""

    def __init__(self):
        self.n = 0
        self.m = {}

    def col(self, name):
        if name not in self.m:
            self.m[name] = self.n
            self.n += 1
        return self.m[name]


def build_nc(b_pc, finalize=True):
    ng, sg_sizes, d_chunks = _cfg(b_pc)
    n_sg = len(sg_sizes)
    nc = bacc.Bacc(trn_type="TRN2") if finalize else bass.Bass(trn_type="TRN2")

    pred_in = nc.dram_tensor("pred_in", [b_pc, C, HW], F32, kind="ExternalInput")
    t_in = nc.dram_tensor("t_in", [b_pc, HW], I32, kind="ExternalInput")
    i_in = nc.dram_tensor("i_in", [b_pc, HW], I32, kind="ExternalInput")
    strat_in = nc.dram_tensor("strat_in", [128, 512], F32, kind="ExternalInput")

    cm = ColMap()
    for sg in range(n_sg):
        for ph in range(len(d_chunks)):
            cm.col(f"fs_{sg}_{ph}")
            cm.col(f"iou_{sg}_{ph}")
            cm.col(f"eqc_{sg}_{ph}")
        cm.col(f"th_{sg}")
        cm.col(f"tv_{sg}")
        cm.col(f"d_{sg}")
    for g in range(ng):
        for k in range(3):
            cm.col(f"cnt3_{g}_{k}")
    ncols = cm.n

    out_cols = nc.dram_tensor("out_cols", [P, ncols], F32, kind="ExternalOutput")
    out_strat = nc.dram_tensor("out_strat", [128, 1], F32, kind="ExternalOutput")

    # ---- inline constants ----
    sw = np.repeat(_spatial_weights().reshape(1, HW), P, axis=0).astype(np.float16)
    sw_const = nc.inline_tensor(sw, name="sw_const")                     # [P, HW]
    iota = (np.arange(P) % C).astype(np.float32).reshape(P, 1)
    iota_const = nc.inline_tensor(iota, name="iota_const")               # [P, 1]
    # channel-sum one-hot lhsT: L[p=(b,c), gl*P + gl*BG+b] = 1  (k=group tile
    # partitions, cols = packed row q = gl*BG+b)
    lhs = np.zeros((P, 10 * P), dtype=np.float16)
    # broadcast lhsT: B[k=q=(gl*BG+b), gl*P + b*C+c] = 1 (k=packed rows,
    # cols = group tile partition p)
    bca = np.zeros((P, 10 * P), dtype=np.float16)
    for gl in range(10):
        for b in range(BG):
            for c in range(C):
                lhs[b * C + c, gl * P + gl * BG + b] = 1.0
                bca[gl * BG + b, gl * P + b * C + c] = 1.0
    lhs_const = nc.inline_tensor(lhs, name="lhs_const")                  # [P, 10*P]
    bca_const = nc.inline_tensor(bca, name="bca_const")                  # [P, 10*P]

    with tile.TileContext(nc) as tc, ExitStack() as es:
        _emit(es, tc, nc, cm, sg_sizes, d_chunks,
              pred_in, t_in, i_in, strat_in, out_cols, out_strat,
              sw_const, iota_const, lhs_const, bca_const)
    if finalize:
        nc.finalize()
    return nc, cm, sg_sizes, d_chunks


def _emit(es, tc, nc, cm, sg_sizes, d_chunks,
          pred_in, t_in, i_in, strat_in, out_cols, out_strat,
          sw_const, iota_const, lhs_const, bca_const):
    dma = nc.sync.dma_start
    stt = nc.vector.scalar_tensor_tensor

    singles = es.enter_context(tc.tile_pool(name="singles", bufs=1))
    xpool = es.enter_context(tc.tile_pool(name="xpool", bufs=2))
    epool = es.enter_context(tc.tile_pool(name="epool", bufs=1))
    mpool = es.enter_context(tc.tile_pool(name="mpool", bufs=1))
    tpool = es.enter_context(tc.tile_pool(name="tpool", bufs=1))
    small = es.enter_context(tc.tile_pool(name="small", bufs=1))
    scr = es.enter_context(tc.tile_pool(name="scr", bufs=2))
    reps = es.enter_context(tc.tile_pool(name="reps", bufs=3))
    dpool = es.enter_context(tc.tile_pool(name="dpool", bufs=1, space="DRAM"))
    psum = es.enter_context(tc.tile_pool(name="psum", bufs=1, space="PSUM"))

    # resident constants + staged output columns
    sw_t = singles.tile([P, HW], F16)
    dma(out=sw_t[:], in_=sw_const[:, :])
    iota_t = singles.tile([P, 1], F32)
    dma(out=iota_t[:], in_=iota_const[:, :])
    lhs_t = singles.tile([P, 10 * P], F16)
    dma(out=lhs_t[:], in_=lhs_const[:, :])
    bca_t = singles.tile([P, 10 * P], F16)
    dma(out=bca_t[:], in_=bca_const[:, :])
    colstage = singles.tile([P, cm.n], F32)
    nc.vector.memset(colstage[:], 0.0)
    # absorb const-DMA queue sems into the DVE vector clock so downstream
    # compute ops need <=2 sync waits (walrus STT struct limit)
    absorb = singles.tile([P, 4], F32, tag="absorb")
    nc.vector.tensor_copy(absorb[:, 0:1], iota_t[:, 0:1])
    nc.vector.tensor_copy(absorb[:, 1:2], sw_t[:, 0:1])
    nc.vector.tensor_copy(absorb[:, 2:3], lhs_t[:, 0:1])
    nc.vector.tensor_copy(absorb[:, 3:4], bca_t[:, 0:1])

    # strategic partial sums
    strat_sb = singles.tile([128, 512], F32, tag="strat")
    dma(out=strat_sb[:], in_=strat_in[:, :])
    strat_col = singles.tile([128, 1], F32, tag="strat_col")
    nc.vector.reduce_sum(strat_col[:], strat_sb[:], axis=X_AXIS)
    dma(out=out_strat[:, :], in_=strat_col[:])

    def ccol(name, r):
        return colstage[:r, cm.col(name):cm.col(name) + 1]

    g0 = 0
    for sg, G in enumerate(sg_sizes):
        R = G * BG                                   # packed rows this supergroup
        # ---- packed int tiles: targets / inputs ----
        t_pk = tpool.tile([P, HW], I32, tag="t_pk")
        i_pk = tpool.tile([P, HW], I32, tag="i_pk")
        dma(out=t_pk[:R], in_=t_in[g0 * BG:g0 * BG + R, :])
        dma(out=i_pk[:R], in_=i_in[g0 * BG:g0 * BG + R, :])
        t16 = tpool.tile([P, HW], F16, tag="t16")
        nc.vector.tensor_copy(t16[:R], t_pk[:R])

        # transitions (horizontal / vertical) + [t==inputs] count
        scratch = scr.tile([P, HW], F16, tag="scratch")
        t3 = t_pk.rearrange("p (h w) -> p h w", h=H)
        s3 = scratch.rearrange("p (h w) -> p h w", h=H)
        stt(out=s3[:R, :, 0:W - 1], in0=t3[:R, :, 0:W - 1], scalar=0.0,
            in1=t3[:R, :, 1:W], op0=OP.bypass, op1=OP.not_equal,
            accum_out=ccol(f"th_{sg}", R))
        stt(out=s3[:R, 0:H - 1, :], in0=t3[:R, 0:H - 1, :], scalar=0.0,
            in1=t3[:R, 1:H, :], op0=OP.bypass, op1=OP.not_equal,
            accum_out=ccol(f"tv_{sg}", R))
        stt(out=scratch[:R], in0=t_pk[:R], scalar=0.0, in1=i_pk[:R],
            op0=OP.bypass, op1=OP.is_equal, accum_out=ccol(f"d_{sg}", R))

        # ---- per-group: load, exp, PE-broadcast targets, mask ----
        e_tiles = []
        m_tiles = []
        epv_dr = dpool.tile([P, HW], F16, tag="epvd")
        for gl in range(G):
            g = g0 + gl
            x_t = xpool.tile([P, HW], F32, tag="x")
            src = pred_in[g * BG:(g + 1) * BG, :, :].rearrange("b c s -> (b c) s")
            dma(out=x_t[:], in_=src)
            e_t = epool.tile([P, HW], F16, tag=f"e{gl}")
            nc.scalar.activation(e_t[:], x_t[:], AF.Exp)

            me_t = mpool.tile([P, HW], F16, tag=f"m{gl}")
            bcl = bca_t[0:R, gl * P:(gl + 1) * P]
            for h0 in range(0, HW, 1024):
                hn = min(1024, HW - h0)
                trep = psum.tile([P, 1024], F32, tag="bcast")
                for k0 in range(0, hn, MM_N):
                    kn = min(MM_N, hn - k0)
                    nc.tensor.matmul(
                        trep[:, k0:k0 + kn], bcl,
                        t16[0:R, h0 + k0:h0 + k0 + kn],
                        start=True, stop=True)
                stt(out=me_t[:, h0:h0 + hn], in0=trep[:, :hn],
                    scalar=iota_t[:], in1=e_t[:, h0:h0 + hn],
                    op0=OP.is_equal, op1=OP.mult,
                    accum_out=ccol(f"cnt3_{g}_{h0 // 1024}", P))
            e_tiles.append(e_t)
            m_tiles.append(me_t)

        # ---- per D-chunk phase: PE sums, packed math ----
        for ph, (d0, D) in enumerate(d_chunks):
            epv_ps = psum.tile([P, D], F32, tag="epv")
            sum_ps = psum.tile([P, D], F32, tag="sum")
            for gl in range(G):
                lw = lhs_t[:, gl * P:gl * P + R]
                for k0 in range(0, D, MM_N):
                    kn = min(MM_N, D - k0)
                    nc.tensor.matmul(
                        epv_ps[0:R, k0:k0 + kn], lw,
                        m_tiles[gl][:, d0 + k0:d0 + k0 + kn],
                        start=(gl == 0), stop=(gl == G - 1))
                    nc.tensor.matmul(
                        sum_ps[0:R, k0:k0 + kn], lw,
                        e_tiles[gl][:, d0 + k0:d0 + k0 + kn],
                        start=(gl == 0), stop=(gl == G - 1))

            log_s = small.tile([P, 1024], F32, tag="log_s")
            nc.scalar.activation(log_s[:R, :D], sum_ps[0:R], AF.Ln)
            log_pv = small.tile([P, 1024], F32, tag="log_pv")
            nc.scalar.activation(log_pv[:R, :D], epv_ps[0:R], AF.Ln)
            epv16 = small.tile([P, 1024], F16, tag="epv16")
            nc.vector.tensor_copy(epv16[:R, :D], epv_ps[0:R])
            dma(out=epv_dr[:R, d0:d0 + D], in_=epv16[:R, :D])

            gcnt_ps = psum.tile([P, D], F32, tag="gcnt")
            for gl in range(G):
                rep_sb = reps.tile([P, 1024], F16, tag="rep_sb")
                base = epv_dr[gl * BG:(gl + 1) * BG, d0:d0 + D]
                rep_src = bass.AP(tensor=base.tensor, offset=base.offset,
                                  ap=[base.ap[0], [0, C], base.ap[1]])
                dma(out=rep_sb[:, :D], in_=rep_src)
                geq = scr.tile([P, 1024], F16, tag="geq")
                nc.vector.tensor_tensor(
                    out=geq[:, :D], in0=e_tiles[gl][:, d0:d0 + D],
                    in1=rep_sb[:, :D], op=OP.is_ge)
                lw = lhs_t[:, gl * P:gl * P + R]
                for k0 in range(0, D, MM_N):
                    kn = min(MM_N, D - k0)
                    nc.tensor.matmul(
                        gcnt_ps[0:R, k0:k0 + kn], lw, geq[:, k0:k0 + kn],
                        start=(gl == 0), stop=(gl == G - 1))

            eq = small.tile([P, 1024], F16, tag="eq")
            nc.vector.tensor_scalar(
                out=eq[:R, :D], in0=gcnt_ps[0:R], scalar1=1.0, scalar2=None,
                op0=OP.is_equal, op1=OP.add, accum_out=ccol(f"eqc_{sg}_{ph}", R))
            iou_scr = small.tile([P, 1024], F16, tag="iou_scr")
            stt(out=iou_scr[:R, :D], in0=eq[:R, :D], scalar=0.0,
                in1=sw_t[:R, d0:d0 + D], op0=OP.bypass, op1=OP.mult,
                accum_out=ccol(f"iou_{sg}_{ph}", R))

            ce = small.tile([P, 1024], F32, tag="ce")
            nc.gpsimd.tensor_tensor(out=ce[:R, :D], in0=log_s[:R, :D],
                                    in1=log_pv[:R, :D], op=OP.subtract)
            pt = small.tile([P, 1024], F32, tag="pt")
            nc.scalar.activation(pt[:R, :D], ce[:R, :D], AF.Exp, scale=-1.0)
            u = small.tile([P, 1024], F32, tag="u")
            nc.scalar.activation(u[:R, :D], pt[:R, :D], AF.Relu,
                                 bias=1.0, scale=-1.0)
            lu = small.tile([P, 1024], F32, tag="lu")
            nc.scalar.activation(lu[:R, :D], u[:R, :D], AF.Ln)
            p25 = small.tile([P, 1024], F32, tag="p25")
            nc.scalar.activation(p25[:R, :D], lu[:R, :D], AF.Exp, scale=2.5)
            fs_scr = small.tile([P, 1024], F32, tag="fs_scr")
            stt(out=fs_scr[:R, :D], in0=p25[:R, :D], scalar=0.0,
                in1=ce[:R, :D], op0=OP.bypass, op1=OP.mult,
                accum_out=ccol(f"fs_{sg}_{ph}", R))

        g0 += G

    dma(out=out_cols[:, :], in_=colstage[:])


_NC_CACHE = {}


def _get_nc(b_pc):
    if b_pc not in _NC_CACHE:
        _NC_CACHE[b_pc] = build_nc(b_pc)
    return _NC_CACHE[b_pc]


def _combine(res_list, cm, sg_sizes, d_chunks, b_pc, bpc_real,
             pred, targets, inputs_arr, sf, ps, rd):
    """host-side final reduction from per-core column outputs"""
    ng = b_pc // BG
    n_sg = len(sg_sizes)
    B = pred.shape[0]
    n_cores = len(res_list)

    fsum = np.zeros(B, np.float64)
    iou_s = np.zeros(B, np.float64)
    eqc = np.zeros(B, np.float64)
    uniq = np.zeros(B, np.int64)
    trans = np.zeros(B, np.int64)
    dcnt = np.zeros(B, np.int64)

    for core, r in enumerate(res_list):
        cols = r["out_cols"]                        # [P, ncols]
        sl0 = core * bpc_real
        g0 = 0
        for sg, G in enumerate(sg_sizes):
            R = G * BG
            rows = np.arange(R)
            gb = g0 * BG + rows                     # per-core padded batch idx
            valid = gb < bpc_real
            bidx = sl0 + gb[valid]
            fs = np.zeros(R); io = np.zeros(R); ec = np.zeros(R)
            for ph in range(len(d_chunks)):
                fs += cols[:R, cm.col(f"fs_{sg}_{ph}")]
                io += cols[:R, cm.col(f"iou_{sg}_{ph}")]
                ec += cols[:R, cm.col(f"eqc_{sg}_{ph}")]
            fsum[bidx] = fs[valid]
            iou_s[bidx] = io[valid]
            eqc[bidx] = ec[valid]
            trans[bidx] = np.rint(cols[:R, cm.col(f"th_{sg}")]
                                  + cols[:R, cm.col(f"tv_{sg}")])[valid]
            dcnt[bidx] = np.rint(cols[:R, cm.col(f"d_{sg}")])[valid]
            g0 += G
        # presence: cnt3 columns, rows p=(b,c)
        for g in range(ng):
            cnt = sum(cols[:, cm.col(f"cnt3_{g}_{k}")] for k in range(3))
            pres = (cnt.reshape(BG, C) > 0).sum(1)             # [b]
            gb = g * BG + np.arange(BG)
            valid = gb < bpc_real
            uniq[sl0 + gb[valid]] = pres[valid]

    sw64 = _spatial_weights().astype(np.float64)
    SW = sw64.sum()
    w = np.where(uniq > 4, 1.3, 1.0) * np.where(trans > W, 1.2, 1.0)
    focal = (fsum * w).sum() / (B * HW)

    strict = eqc == HW
    iou = iou_s / SW
    ut = 0.85 * iou + 0.15 * strict
    ut_mean = ut.mean()
    exact_bonus = max(-ut_mean * 5.0, -5.0)

    cand = np.where(eqc == dcnt)[0]
    copy = np.zeros(B, np.float64)
    if cand.size:
        pr = pred.reshape(B, C, HW)
        am = pr[cand].argmax(1)
        copy[cand] = (am == inputs_arr.reshape(B, HW)[cand]).all(1)
    transform_penalty = copy.mean() * 0.5

    strat_total = sum(float(r["out_strat"].sum()) for r in res_list)
    sf_mean = strat_total / sf.size
    creativity = 1.0 / (1.0 + np.exp(-sf_mean)) * 0.1
    strategic = ps.astype(np.float64).mean() * 0.1
    multi = rd.astype(np.float64).mean() * 0.1
    complexity = ut_mean * (HW / 1225.0) * 0.1

    total = (focal + transform_penalty + exact_bonus
             - creativity - strategic - multi - complexity)
    if np.isnan(total) or np.isinf(total):
        total = min(focal, 10.0)
    return np.float32(total)


def kernel(pred, strategic_features, planning_score, reasoning_depth,
           targets, inputs):
    global LAST_EXEC_NS
    pred = np.ascontiguousarray(np.asarray(pred, dtype=np.float32))
    targets = np.ascontiguousarray(np.asarray(targets, dtype=np.int32))
    inputs_arr = np.ascontiguousarray(np.asarray(inputs, dtype=np.int32))
    sf = np.asarray(strategic_features, dtype=np.float32)
    ps = np.asarray(planning_score, dtype=np.float32)
    rd = np.asarray(reasoning_depth, dtype=np.float32)

    B = pred.shape[0]
    bpc = B // N_CORES                                 # 256
    b_pc = ((bpc + BG - 1) // BG) * BG
    if (b_pc // BG) % 2:                               # keep 22 groups for 256
        pass
    nc, cm, sg_sizes, d_chunks = _get_nc(b_pc if bpc % BG == 0 else bpc + (BG - bpc % BG))
    b_pc = bpc + (BG - bpc % BG) % BG

    in_maps = []
    for core in range(N_CORES):
        sl = slice(core * bpc, (core + 1) * bpc)
        p_c = pred[sl].reshape(bpc, C, HW)
        t_c = targets[sl].reshape(bpc, HW)
        i_c = inputs_arr[sl].reshape(bpc, HW)
        pad = b_pc - bpc
        if pad:
            p_c = np.concatenate([p_c, np.broadcast_to(p_c[:1], (pad, C, HW))], 0)
            t_c = np.concatenate([t_c, np.broadcast_to(t_c[:1], (pad, HW))], 0)
            i_c = np.concatenate([i_c, np.broadcast_to(i_c[:1], (pad, HW))], 0)
        in_maps.append({
            "pred_in": np.ascontiguousarray(p_c),
            "t_in": np.ascontiguousarray(t_c),
            "i_in": np.ascontiguousarray(i_c),
            "strat_in": np.ascontiguousarray(sf[sl].reshape(128, 512)),
        })

    trace = os.environ.get("BASSLOSS_TRACE", "0") == "1"
    res = run_bass_kernel_spmd(nc, in_maps, list(range(N_CORES)), trace=trace)
    LAST_EXEC_NS = res.exec_time_ns

    return _combine(res.results, cm, sg_sizes, d_chunks, b_pc, bpc,
                    pred, targets, inputs_arr, sf, ps, rd)


if __name__ == "__main__":
    d = np.load("/root/problem/inputs_cache.npz")
    out = kernel(**{k: d[k] for k in d.files})
    print("kernel out:", out, " exec_ns:", LAST_EXEC_NS)
